# revision 44
# baseline (speedup 1.0000x reference)
"""Trainium2 Bass kernel for nn_Eq2to2 (permutation-equivariant 2->2 layer).

Math (per batch n, M=128, D=S=64, derived from the 15-basis eops decomposition):
  out[i,j,s] = leaky_relu( X[i,j,:]@C0 + X[j,i,:]@C1 + P[i,s] + Q[j,s] + diag_ij*Dg[i,s] )
  with per-index features diag/rowsum/colsum and scalars sum_diag/sum_all
  contracted against coef slices C2..C14 (+bias, diag_bias) into Q, P, Dg.
  (mask is handled on host; it is all-ones in the spec.)

Sharding: pure data parallel; batch n -> core n (N=8, 8 cores).

Layouts on device (per core), all on SBUF partitions 0-63 unless noted:
  X wave  [128, 2048] : X[i, j*64+d] for one 32-j wave (i on partitions)
  XT2e    [64, 8192]  : X[i, 2k,   d] at [d, k*128 + i]
  XT2o    [64, 8192]  : X[i, 2k+1, d] at [d, k*128 + i]
  zz      [64, 16384] : z[s, j*128 + i]  (pre-activation)
  out_sb  [128, 8192] : out[i, j*64+s]   (natural)

fp32r is used for the big matmuls (1 cyc/row at N>=256). Constraints learned on
hardware: fp32r operands must be produced rounded (bitcast F32R on the producer
out AP); fp32r supports only PE row tiling; and the PE row position must not
change within a psum accumulation group -> everything here runs at row 0.
"""

import os
import sys

import numpy as np

sys.path.insert(0, "/opt/trn_rl_repo")

import concourse.bass as bass
import concourse.bacc as bacc
import concourse.tile as tile
from concourse import mybir
from concourse.masks import make_identity

F32 = mybir.dt.float32
F32R = mybir.dt.float32r
AX = mybir.AxisListType
ALU = mybir.AluOpType

M = 128          # objects per event (i, j)
D = 64           # input channels
S = 64           # output channels
NB = 15          # basis size
NCORES = 8
NWAVE = 4        # DMA / pipeline waves
PAIRS = M // 2   # 64 j-pairs


def _ap(base, free_dims):
    """Raw AP with base's partition dim + custom free [step, count] dims."""
    return bass.AP(tensor=base.tensor, offset=base.offset,
                   ap=[list(base.ap[0])] + [list(d) for d in free_dims])


def build_nc(debug_stage=None, until=None):
    nc = bacc.Bacc(None, target_bir_lowering=False)

    x_d = nc.declare_dram_parameter("x", [M, M * D], F32, isOutput=False)
    coefs_d = nc.declare_dram_parameter("coefs", [D, S * NB], F32, isOutput=False)
    bias_d = nc.declare_dram_parameter("biasv", [S, 1], F32, isOutput=False)
    dbias_d = nc.declare_dram_parameter("dbiasv", [S, 1], F32, isOutput=False)
    out_d = nc.declare_dram_parameter("out", [M, M * S], F32, isOutput=True)
    dbg_d = (nc.declare_dram_parameter("dbg", [M, M * S], F32, isOutput=True)
             if debug_stage else None)

    with tile.TileContext(nc) as tc:
        with (
            tc.tile_pool(name="big", bufs=1) as big,
            tc.tile_pool(name="xw", bufs=2) as xw,
            tc.tile_pool(name="pT", bufs=2, space="PSUM") as pT,
            tc.tile_pool(name="pz", bufs=4, space="PSUM") as pz,
            tc.tile_pool(name="pt", bufs=2, space="PSUM") as pt,
        ):
            # ---------------- persistent SBUF ----------------
            XT2e = big.tile([D, PAIRS * M], F32, tag="XT2e")
            XT2o = big.tile([D, PAIRS * M], F32, tag="XT2o")
            zz = big.tile([S, M * M], F32, tag="zz")
            out_sb = big.tile([M, M * S], F32, tag="out_sb")
            coefs_sb = big.tile([D, S * NB], F32, tag="coefs_sb")
            ident = big.tile([M, M], F32, tag="ident")
            bias_sb = big.tile([S, 1], F32, tag="bias_sb")
            dbias_sb = big.tile([S, 1], F32, tag="dbias_sb")
            colsumE = big.tile([D, PAIRS], F32, tag="colsumE")   # colsum[d, 2k]
            colsumO = big.tile([D, PAIRS], F32, tag="colsumO")   # colsum[d, 2k+1]
            rowsumT = big.tile([D, M], F32, tag="rowsumT")       # rowsum[d, t]
            rswE = big.tile([D, M], F32, tag="rswE")             # per-wave partials
            rswO = big.tile([D, M], F32, tag="rswO")
            diagE = big.tile([D, PAIRS], F32, tag="diagE")       # diag[d, 2k]
            diagO = big.tile([D, PAIRS], F32, tag="diagO")
            sdV = big.tile([D, 1], F32, tag="sdV")               # sum_diag
            saV = big.tile([D, 1], F32, tag="saV")               # sum_all
            tmp1 = big.tile([D, 1], F32, tag="tmp1")
            QT = big.tile([S, M], F32, tag="QT")                 # [s, j] natural
            DT = big.tile([S, M], F32, tag="DT")                 # [s, j] natural
            PT_sb = big.tile([S, M], F32, tag="PT_sb")           # [s, t] natural
            P_is = big.tile([M, S], F32, tag="P_is")             # [t, s]
            cQ = big.tile([S, 1], F32, tag="cQ")
            cD = big.tile([S, 1], F32, tag="cD")
            identr_t = big.tile([M, M], F32, tag="identr_t")
            c0r_t = big.tile([D, S], F32, tag="c0r_t")
            c1r_t = big.tile([D, S], F32, tag="c1r_t")

            make_identity(nc, ident[:, :])
            # f32r-rounded copies for fp32r matmul operands (walrus requires
            # producers of fp32r matmul inputs to round their outputs)
            nc.vector.tensor_copy(identr_t.bitcast(F32R), ident[:, :])
            identr = identr_t.bitcast(F32R)

            nc.sync.dma_start(out=coefs_sb[:, :], in_=coefs_d[:, :])
            nc.sync.dma_start(out=bias_sb[:, :], in_=bias_d[:, :])
            nc.sync.dma_start(out=dbias_sb[:, :], in_=dbias_d[:, :])

            c3 = coefs_sb.rearrange("p (s b) -> p b s", b=NB)  # [64, 15, 64]

            def Cs(b):
                return c3[:, b, :]

            # ---------------- input: DMA + transposes + reduces, per wave ----------------
            WJ = M // NWAVE                 # 32 j per wave
            WP = PAIRS // NWAVE             # 16 pairs per wave
            for w in range(NWAVE):
                xt = xw.tile([M, WJ * D], F32, tag="xw")
                nc.sync.dma_start(out=xt[:, :],
                                  in_=x_d[:, w * WJ * D:(w + 1) * WJ * D])
                # 32 single-slab transposes -> 8 psum tiles, 4 same-parity j each
                for g in range(8):
                    par, blk = g % 2, g // 2
                    ptile = pT.tile([D, 512], F32, tag="pT")
                    dst = XT2o if par else XT2e
                    for q in range(4):
                        jl = blk * 8 + 2 * q + par       # j within wave
                        nc.tensor.transpose(
                            ptile[:, q * M:(q + 1) * M],
                            xt[:, jl * D:(jl + 1) * D],
                            ident[:, :],
                        )
                    kbase = w * WP + blk * 4             # pair-block index
                    nc.scalar.copy(out=dst[:, kbase * M:kbase * M + 512].bitcast(F32R),
                                   in_=ptile[:, :])

                # per-wave reduces over this wave's XT2 spans [64, WP*128]
                for src, cs, rsw in ((XT2e, colsumE, rswE), (XT2o, colsumO, rswO)):
                    xv = src[:, w * WP * M:(w + 1) * WP * M]
                    x3 = xv.rearrange("p (k i) -> p k i", i=M)       # [64, 16, 128]
                    nc.vector.tensor_reduce(out=cs[:, w * WP:(w + 1) * WP], in_=x3,
                                            axis=AX.X, op=ALU.add)
                    nc.vector.tensor_reduce(out=rsw[:, :], in_=x3.transpose([0, 2, 1]),
                                            axis=AX.X, op=ALU.add)
                if w == 0:
                    nc.vector.tensor_add(rowsumT[:, :], rswE[:, :], rswO[:, :])
                else:
                    nc.vector.tensor_add(rowsumT[:, :], rowsumT[:, :], rswE[:, :])
                    nc.vector.tensor_add(rowsumT[:, :], rowsumT[:, :], rswO[:, :])
                # diag slices: diagE[d,k]=XT2e[d,130k]; diagO[d,k]=XT2o[d,130k+1]
                nc.vector.tensor_copy(diagE[:, w * WP:(w + 1) * WP],
                                      _ap(XT2e[:, w * WP * 130:], [[130, WP]]))
                nc.vector.tensor_copy(diagO[:, w * WP:(w + 1) * WP],
                                      _ap(XT2o[:, w * WP * 130 + 1:], [[130, WP]]))

            if until == "in":
                nc.sync.dma_start(out=out_d[0:D, 0:8192], in_=XT2e[:, :])
                nc.sync.dma_start(out=out_d[D:2 * D, 0:8192], in_=XT2o[:, :])
                return nc

            nc.vector.tensor_reduce(out=sdV[:, :], in_=diagE[:, :], axis=AX.X, op=ALU.add)
            nc.vector.tensor_reduce(out=tmp1[:, :], in_=diagO[:, :], axis=AX.X, op=ALU.add)
            nc.vector.tensor_add(sdV[:, :], sdV[:, :], tmp1[:, :])
            nc.vector.tensor_reduce(out=saV[:, :], in_=rowsumT[:, :], axis=AX.X, op=ALU.add)

            if until == "reduce":
                nc.sync.dma_start(out=out_d[0:D, 0:M], in_=rowsumT[:, :])
                nc.sync.dma_start(out=out_d[0:D, M:M + PAIRS], in_=colsumE[:, :])
                nc.sync.dma_start(out=out_d[0:D, 256:256 + PAIRS], in_=colsumO[:, :])
                nc.sync.dma_start(out=out_d[0:D, 512:512 + PAIRS], in_=diagE[:, :])
                return nc

            # ---------------- small matmuls: Q, D, P fields (all row 0) ----------------
            # field psums use parity-blocked cols (par*64+k); rowsum rhs reordered to match
            rs_pb = rowsumT.rearrange("p (k par) -> p par k", par=2)  # [64, 2, 64]

            def fold_mms(psum_t, cb_sum_diag, cb_sum_all, vec_bias):
                """psum[s, 0] <- C_a^T sum_diag + C_b^T sum_all + I*bias."""
                nc.tensor.matmul(psum_t, Cs(cb_sum_diag), sdV[:, :],
                                 start=True, stop=False, skip_group_check=True)
                nc.tensor.matmul(psum_t, Cs(cb_sum_all), saV[:, :],
                                 start=False, stop=False, skip_group_check=True)
                nc.tensor.matmul(psum_t, ident[0:S, 0:S], vec_bias[:, :],
                                 start=False, stop=True, skip_group_check=True)

            def field_mms(psum_t, cb_diag, cb_col, cb_row):
                """psum[s, par*64+k] <- (C_d^T diag + C_c^T colsum + C_r^T rowsum)[j=2k+par]."""
                oe = psum_t[:, 0:PAIRS]
                oo = psum_t[:, PAIRS:2 * PAIRS]
                # full-span mm first with start=True (psum zeroing is bank-granular)
                nc.tensor.matmul(psum_t, Cs(cb_row), rs_pb,
                                 start=True, stop=False, skip_group_check=True)
                nc.tensor.matmul(oe, Cs(cb_diag), diagE[:, :],
                                 start=False, stop=False, skip_group_check=True)
                nc.tensor.matmul(oo, Cs(cb_diag), diagO[:, :],
                                 start=False, stop=False, skip_group_check=True)
                nc.tensor.matmul(oe, Cs(cb_col), colsumE[:, :],
                                 start=False, stop=True, skip_group_check=True)
                nc.tensor.matmul(oo, Cs(cb_col), colsumO[:, :],
                                 start=False, stop=True, skip_group_check=True)

            def unperm(pb):
                # parity-blocked cols (par*64+k) -> natural (2k+par), as a read view
                return pb.rearrange("p (par k) -> p k par", par=2)

            # small psums share the pt pool (outT only starts much later)
            pc = pt.tile([M, 512], F32, tag="pt")
            fold_mms(pc[0:S, 0:1], 13, 14, bias_sb)
            fold_mms(pc[0:S, 1:2], 5, 8, dbias_sb)
            nc.vector.tensor_copy(cQ[:, :], pc[0:S, 0:1])
            nc.vector.tensor_copy(cD[:, :], pc[0:S, 1:2])
            pq = pt.tile([M, 512], F32, tag="pt")
            field_mms(pq[0:S, 0:M], 3, 9, 10)
            nc.vector.tensor_tensor(QT[:, :], unperm(pq[0:S, 0:M]),
                                    _ap(cQ[:, 0:], [[0, M]]), op=ALU.add)
            pd = pt.tile([M, 512], F32, tag="pt")
            field_mms(pd[0:S, 0:M], 2, 7, 6)
            nc.vector.tensor_tensor(DT[:, :], unperm(pd[0:S, 0:M]),
                                    _ap(cD[:, 0:], [[0, M]]), op=ALU.add)
            pp = pt.tile([M, 512], F32, tag="pt")
            field_mms(pp[0:S, 0:M], 4, 11, 12)
            nc.vector.tensor_copy(PT_sb[:, :], unperm(pp[0:S, 0:M]))
            ppt = pt.tile([M, 512], F32, tag="pt")
            nc.tensor.transpose(ppt[:, 0:S], PT_sb[:, :], ident[0:S, 0:S])
            nc.vector.tensor_copy(P_is.bitcast(F32R), ppt[:, 0:S])
            # contiguous f32r-rounded C0/C1 for the big matmuls
            nc.vector.tensor_copy(c0r_t.bitcast(F32R), Cs(0))
            nc.vector.tensor_copy(c1r_t.bitcast(F32R), Cs(1))

            if until == "small":
                nc.sync.dma_start(out=out_d[0:S, 0:M], in_=QT[:, :])
                nc.sync.dma_start(out=out_d[0:S, M:2 * M], in_=DT[:, :])
                nc.sync.dma_start(out=out_d[0:M, 2 * M:2 * M + S], in_=P_is[:, :])
                return nc

            # ---------------- big matmuls ----------------
            # chunk (c, jp): j in {8c+jp, 8c+2+jp, 8c+4+jp, 8c+6+jp}; psum Z [64, 512]
            # Z free layout: (ip, k, m) -> ip*256 + k*64 + m ; i = 2m+ip
            xe = XT2e.bitcast(F32R)
            xo = XT2o.bitcast(F32R)
            # mm1 rhs stream (ip, pr, m): value X[i=2m+ip, j-of-block, d]
            xe_ipm = xe.rearrange("p (pr m ip) -> p ip pr m", m=S, ip=2)   # [64,2,64,64]
            xo_ipm = xo.rearrange("p (pr m ip) -> p ip pr m", m=S, ip=2)
            # mm2 rhs stream (j2, m): value X[j=8c+2*j2+jp, i-of-block, d]
            xe_mj = xe.rearrange("p (m j2 jp) -> p jp j2 m", j2=S, jp=2)   # [64,2,64,64]
            xo_mj = xo.rearrange("p (m j2 jp) -> p jp j2 m", j2=S, jp=2)
            p_isr = P_is.bitcast(F32R)
            # mmP rhs: indicator delta_{t, i=2m+ip} streamed in (ip, k, m) order
            irep = (identr.rearrange("p (m ip) -> p ip m", ip=2)
                    .unsqueeze(2).broadcast_to([M, 2, 4, S]))
            c0 = c0r_t.bitcast(F32R)
            c1 = c1r_t.bitcast(F32R)

            for cc in range(32):
                c, jp = cc // 2, cc % 2
                xjp = xe_ipm if jp == 0 else xo_ipm
                Z = pz.tile([S, 512], F32, tag="pz")
                # W0: full 512 stream (ip, pr, m)
                nc.tensor.matmul(Z[:, :], c0[:, :], xjp[:, :, 4 * c:4 * c + 4, :],
                                 start=True, stop=False, skip_group_check=True)
                # W1: one mm per i-parity (i-parity selects the XT2 tensor)
                for ip, xmj in ((0, xe_mj), (1, xo_mj)):
                    nc.tensor.matmul(Z[:, ip * 256:(ip + 1) * 256], c1[:, :],
                                     xmj[:, jp, 4 * c:4 * c + 4, :],
                                     start=False, stop=False, skip_group_check=True)
                # P term: + P[i, s] everywhere (indicator rhs, K=128)
                nc.tensor.matmul(Z[:, :], p_isr, irep,
                                 start=False, stop=True, skip_group_check=True)
                # psum -> zz with un-permute: zz col = (8c+2k+jp)*128 + 2m + ip
                zzv = bass.AP(
                    tensor=zz.tensor, offset=zz[:, (8 * c + jp) * M:].offset,
                    ap=[list(zz.ap[0]), [1, 2], [2 * M, 4], [2, S]],
                )
                nc.scalar.copy(out=zzv, in_=Z[:, :])

            if until == "big":
                nc.sync.dma_start(out=out_d[0:S, 0:8192], in_=zz[:, 0:8192])
                nc.sync.dma_start(out=out_d[S:2 * S, 0:8192], in_=zz[:, 8192:16384])
                return nc

            # ---------------- epilogue on zz: +Q, +diag, then transpose+leakyrelu ----------------
            use_dve_q = bool(os.environ.get("KQ_DVE"))
            for w in range(NWAVE):
                sl = slice(w * WJ * M, (w + 1) * WJ * M)
                qte = (QT[:, w * WJ:(w + 1) * WJ]
                       .unsqueeze(2).broadcast_to([S, WJ, M]))
                qeng = nc.vector if use_dve_q else nc.gpsimd
                qeng.tensor_tensor(zz[:, sl], zz[:, sl], qte, op=ALU.add)
                # diag: col j*128 + j = 129*j
                de_o = _ap(zz[:, w * WJ * 129:], [[129, WJ]])
                dte = DT[:, w * WJ:(w + 1) * WJ]
                nc.vector.tensor_tensor(de_o, de_o, dte, op=ALU.add)
                # output transposes: 32 cols -> 4 psum tiles of 8; leaky-relu on copy out
                for g in range(4):
                    ttile = pt.tile([M, 512], F32, tag="pt")
                    for q in range(8):
                        j = w * WJ + g * 8 + q
                        nc.tensor.transpose(
                            ttile[:, q * S:(q + 1) * S],
                            zz[:, j * M:(j + 1) * M],
                            ident[0:S, 0:S],
                        )
                    base = (w * WJ + g * 8) * S
                    osl = out_sb[:, base:base + 512]
                    nc.vector.tensor_scalar_mul(osl, ttile[:, :], 0.01)
                    nc.vector.tensor_max(osl, osl, ttile[:, :])
                nc.sync.dma_start(
                    out=out_d[:, w * WJ * S:(w + 1) * WJ * S],
                    in_=out_sb[:, w * WJ * S:(w + 1) * WJ * S],
                )

    return nc


_nc_cache = None


def _get_nc():
    global _nc_cache
    if _nc_cache is None:
        _nc_cache = build_nc()
        _nc_cache.compile()
    return _nc_cache


def run_on_cores(inputs, coefs, bias, diag_bias, **spmd_kwargs):
    """Run the SPMD kernel on the 8 cores; returns (out [8,M,M,S], BassKernelResults)."""
    from concourse.bass_utils import run_bass_kernel_spmd

    inputs = np.ascontiguousarray(np.asarray(inputs, dtype=np.float32))
    coefs = np.asarray(coefs, dtype=np.float32)

    # coefs [D, S, B] -> [D, S*B] row-major (free = s*15+b matches kernel views)
    coefs_flat = np.ascontiguousarray(coefs.reshape(D, S * NB))
    bias_col = np.ascontiguousarray(np.asarray(bias, dtype=np.float32).reshape(S, 1))
    dbias_col = np.ascontiguousarray(np.asarray(diag_bias, dtype=np.float32).reshape(S, 1))

    in_maps = []
    for n in range(NCORES):
        in_maps.append({
            "x": np.ascontiguousarray(inputs[n].reshape(M, M * D)),
            "coefs": coefs_flat,
            "biasv": bias_col,
            "dbiasv": dbias_col,
        })

    nc = _get_nc()
    res = run_bass_kernel_spmd(nc, in_maps, list(range(NCORES)), **spmd_kwargs)
    outs = [np.asarray(res.results[n]["out"]).reshape(M, M, S) for n in range(NCORES)]
    return np.stack(outs, axis=0).astype(np.float32), res


def kernel(inputs, nobj, mask, coefs, bias, diag_bias):
    mask = np.asarray(mask, dtype=np.float32)
    out, _ = run_on_cores(inputs, coefs, bias, diag_bias)
    if not np.all(mask == 1.0):
        out = out * mask.reshape(out.shape[0], M, M, 1)
    return out


if __name__ == "__main__":
    nc = build_nc()
    print("built ok")


# revision 47
# speedup vs baseline: 650.7785x; 650.7785x over previous
"""Trainium2 Bass kernel for nn_Eq2to2 (permutation-equivariant 2->2 layer).

Math (per batch n, M=128, D=S=64, derived from the 15-basis eops decomposition):
  out[i,j,s] = leaky_relu( X[i,j,:]@C0 + X[j,i,:]@C1 + P[i,s] + Q[j,s] + diag_ij*Dg[i,s] )
  with per-index features diag/rowsum/colsum and scalars sum_diag/sum_all
  contracted against coef slices C2..C14 (+bias, diag_bias) into Q, P, Dg.
  (mask is handled on host; it is all-ones in the spec.)

Sharding: pure data parallel; batch n -> core n (N=8, 8 cores).

Layouts on device (per core), all on SBUF partitions 0-63 unless noted:
  X wave  [128, 2048] : X[i, j*64+d] for one 32-j wave (i on partitions)
  XT2e    [64, 8192]  : X[i, 2k,   d] at [d, k*128 + i]
  XT2o    [64, 8192]  : X[i, 2k+1, d] at [d, k*128 + i]
  zz      [64, 16384] : z[s, j*128 + i]  (pre-activation)
  out_sb  [128, 8192] : out[i, j*64+s]   (natural)

fp32r is used for the big matmuls (1 cyc/row at N>=256). Constraints learned on
hardware: fp32r operands must be produced rounded (bitcast F32R on the producer
out AP); fp32r supports only PE row tiling; and the PE row position must not
change within a psum accumulation group -> everything here runs at row 0.
"""

import os
import sys

import numpy as np

sys.path.insert(0, "/opt/trn_rl_repo")

import concourse.bass as bass
import concourse.bacc as bacc
import concourse.tile as tile
from concourse import mybir
from concourse.masks import make_identity

F32 = mybir.dt.float32
F32R = mybir.dt.float32r
AX = mybir.AxisListType
ALU = mybir.AluOpType

M = 128          # objects per event (i, j)
D = 64           # input channels
S = 64           # output channels
NB = 15          # basis size
NCORES = 8
NWAVE = 4        # DMA / pipeline waves
PAIRS = M // 2   # 64 j-pairs


def _ap(base, free_dims):
    """Raw AP with base's partition dim + custom free [step, count] dims."""
    return bass.AP(tensor=base.tensor, offset=base.offset,
                   ap=[list(base.ap[0])] + [list(d) for d in free_dims])


def build_nc(debug_stage=None, until=None, bench_iters=0):
    nc = bacc.Bacc(None, target_bir_lowering=False)

    x_d = nc.declare_dram_parameter("x", [M, M * D], F32, isOutput=False)
    coefs_d = nc.declare_dram_parameter("coefs", [D, S * NB], F32, isOutput=False)
    bias_d = nc.declare_dram_parameter("biasv", [S, 1], F32, isOutput=False)
    dbias_d = nc.declare_dram_parameter("dbiasv", [S, 1], F32, isOutput=False)
    out_d = nc.declare_dram_parameter("out", [M, M * S], F32, isOutput=True)
    dbg_d = (nc.declare_dram_parameter("dbg", [M, M * S], F32, isOutput=True)
             if debug_stage else None)

    with tile.TileContext(nc) as tc:
        with (
            tc.tile_pool(name="big", bufs=1) as big,
            tc.tile_pool(name="xw", bufs=2) as xw,
            tc.tile_pool(name="pT", bufs=2, space="PSUM") as pT,
            tc.tile_pool(name="pz", bufs=4, space="PSUM") as pz,
            tc.tile_pool(name="pt", bufs=2, space="PSUM") as pt,
        ):
            # ---------------- persistent SBUF ----------------
            XT2e = big.tile([D, PAIRS * M], F32, tag="XT2e")
            XT2o = big.tile([D, PAIRS * M], F32, tag="XT2o")
            zz = big.tile([S, M * M], F32, tag="zz")
            out_sb = big.tile([M, M * S], F32, tag="out_sb")
            coefs_sb = big.tile([D, S * NB], F32, tag="coefs_sb")
            ident = big.tile([M, M], F32, tag="ident")
            bias_sb = big.tile([S, 1], F32, tag="bias_sb")
            dbias_sb = big.tile([S, 1], F32, tag="dbias_sb")
            colsumE = big.tile([D, PAIRS], F32, tag="colsumE")   # colsum[d, 2k]
            colsumO = big.tile([D, PAIRS], F32, tag="colsumO")   # colsum[d, 2k+1]
            rowsumT = big.tile([D, M], F32, tag="rowsumT")       # rowsum[d, t]
            rswE = big.tile([D, M], F32, tag="rswE")             # per-wave partials
            rswO = big.tile([D, M], F32, tag="rswO")
            diagE = big.tile([D, PAIRS], F32, tag="diagE")       # diag[d, 2k]
            diagO = big.tile([D, PAIRS], F32, tag="diagO")
            sdV = big.tile([D, 1], F32, tag="sdV")               # sum_diag
            saV = big.tile([D, 1], F32, tag="saV")               # sum_all
            tmp1 = big.tile([D, 1], F32, tag="tmp1")
            QT = big.tile([S, M], F32, tag="QT")                 # [s, j] natural
            DT = big.tile([S, M], F32, tag="DT")                 # [s, j] natural
            PT_sb = big.tile([S, M], F32, tag="PT_sb")           # [s, t] natural
            P_is = big.tile([M, S], F32, tag="P_is")             # [t, s]
            cQ = big.tile([S, 1], F32, tag="cQ")
            cD = big.tile([S, 1], F32, tag="cD")
            identr_t = big.tile([M, M], F32, tag="identr_t")
            c0r_t = big.tile([D, S], F32, tag="c0r_t")
            c1r_t = big.tile([D, S], F32, tag="c1r_t")

            make_identity(nc, ident[:, :])
            # f32r-rounded copies for fp32r matmul operands (walrus requires
            # producers of fp32r matmul inputs to round their outputs)
            nc.vector.tensor_copy(identr_t.bitcast(F32R), ident[:, :])
            identr = identr_t.bitcast(F32R)

            nc.sync.dma_start(out=coefs_sb[:, :], in_=coefs_d[:, :])
            nc.sync.dma_start(out=bias_sb[:, :], in_=bias_d[:, :])
            nc.sync.dma_start(out=dbias_sb[:, :], in_=dbias_d[:, :])

            c3 = coefs_sb.rearrange("p (s b) -> p b s", b=NB)  # [64, 15, 64]

            def Cs(b):
                return c3[:, b, :]

            # ---------------- input: DMA + transposes + reduces, per wave ----------------
            from contextlib import nullcontext
            loop_cm = (tc.For_i(0, bench_iters, 1) if bench_iters > 1
                       else nullcontext())
            loop_cm.__enter__()

            WJ = M // NWAVE                 # 32 j per wave
            WP = PAIRS // NWAVE             # 16 pairs per wave
            for w in range(NWAVE):
                xt = xw.tile([M, WJ * D], F32, tag="xw")
                nc.sync.dma_start(out=xt[:, :],
                                  in_=x_d[:, w * WJ * D:(w + 1) * WJ * D])
                # 32 single-slab transposes -> 8 psum tiles, 4 same-parity j each
                for g in range(8):
                    par, blk = g % 2, g // 2
                    ptile = pT.tile([D, 512], F32, tag="pT")
                    dst = XT2o if par else XT2e
                    for q in range(4):
                        jl = blk * 8 + 2 * q + par       # j within wave
                        nc.tensor.transpose(
                            ptile[:, q * M:(q + 1) * M],
                            xt[:, jl * D:(jl + 1) * D],
                            ident[:, :],
                        )
                    kbase = w * WP + blk * 4             # pair-block index
                    nc.scalar.copy(out=dst[:, kbase * M:kbase * M + 512].bitcast(F32R),
                                   in_=ptile[:, :])

                # per-wave reduces over this wave's XT2 spans [64, WP*128]
                for src, cs, rsw in ((XT2e, colsumE, rswE), (XT2o, colsumO, rswO)):
                    xv = src[:, w * WP * M:(w + 1) * WP * M]
                    x3 = xv.rearrange("p (k i) -> p k i", i=M)       # [64, 16, 128]
                    nc.vector.tensor_reduce(out=cs[:, w * WP:(w + 1) * WP], in_=x3,
                                            axis=AX.X, op=ALU.add)
                    nc.vector.tensor_reduce(out=rsw[:, :], in_=x3.transpose([0, 2, 1]),
                                            axis=AX.X, op=ALU.add)
                if w == 0:
                    nc.vector.tensor_add(rowsumT[:, :], rswE[:, :], rswO[:, :])
                else:
                    nc.vector.tensor_add(rowsumT[:, :], rowsumT[:, :], rswE[:, :])
                    nc.vector.tensor_add(rowsumT[:, :], rowsumT[:, :], rswO[:, :])
                # diag slices: diagE[d,k]=XT2e[d,130k]; diagO[d,k]=XT2o[d,130k+1]
                nc.vector.tensor_copy(diagE[:, w * WP:(w + 1) * WP],
                                      _ap(XT2e[:, w * WP * 130:], [[130, WP]]))
                nc.vector.tensor_copy(diagO[:, w * WP:(w + 1) * WP],
                                      _ap(XT2o[:, w * WP * 130 + 1:], [[130, WP]]))

            if until == "in":
                nc.sync.dma_start(out=out_d[0:D, 0:8192], in_=XT2e[:, :])
                nc.sync.dma_start(out=out_d[D:2 * D, 0:8192], in_=XT2o[:, :])
                return nc

            nc.vector.tensor_reduce(out=sdV[:, :], in_=diagE[:, :], axis=AX.X, op=ALU.add)
            nc.vector.tensor_reduce(out=tmp1[:, :], in_=diagO[:, :], axis=AX.X, op=ALU.add)
            nc.vector.tensor_add(sdV[:, :], sdV[:, :], tmp1[:, :])
            nc.vector.tensor_reduce(out=saV[:, :], in_=rowsumT[:, :], axis=AX.X, op=ALU.add)

            if until == "reduce":
                nc.sync.dma_start(out=out_d[0:D, 0:M], in_=rowsumT[:, :])
                nc.sync.dma_start(out=out_d[0:D, M:M + PAIRS], in_=colsumE[:, :])
                nc.sync.dma_start(out=out_d[0:D, 256:256 + PAIRS], in_=colsumO[:, :])
                nc.sync.dma_start(out=out_d[0:D, 512:512 + PAIRS], in_=diagE[:, :])
                return nc

            # ---------------- small matmuls: Q, D, P fields (all row 0) ----------------
            # field psums use parity-blocked cols (par*64+k); rowsum rhs reordered to match
            rs_pb = rowsumT.rearrange("p (k par) -> p par k", par=2)  # [64, 2, 64]

            def fold_mms(psum_t, cb_sum_diag, cb_sum_all, vec_bias):
                """psum[s, 0] <- C_a^T sum_diag + C_b^T sum_all + I*bias."""
                nc.tensor.matmul(psum_t, Cs(cb_sum_diag), sdV[:, :],
                                 start=True, stop=False, skip_group_check=True)
                nc.tensor.matmul(psum_t, Cs(cb_sum_all), saV[:, :],
                                 start=False, stop=False, skip_group_check=True)
                nc.tensor.matmul(psum_t, ident[0:S, 0:S], vec_bias[:, :],
                                 start=False, stop=True, skip_group_check=True)

            def field_mms(psum_t, cb_diag, cb_col, cb_row):
                """psum[s, par*64+k] <- (C_d^T diag + C_c^T colsum + C_r^T rowsum)[j=2k+par]."""
                oe = psum_t[:, 0:PAIRS]
                oo = psum_t[:, PAIRS:2 * PAIRS]
                # full-span mm first with start=True (psum zeroing is bank-granular)
                nc.tensor.matmul(psum_t, Cs(cb_row), rs_pb,
                                 start=True, stop=False, skip_group_check=True)
                nc.tensor.matmul(oe, Cs(cb_diag), diagE[:, :],
                                 start=False, stop=False, skip_group_check=True)
                nc.tensor.matmul(oo, Cs(cb_diag), diagO[:, :],
                                 start=False, stop=False, skip_group_check=True)
                nc.tensor.matmul(oe, Cs(cb_col), colsumE[:, :],
                                 start=False, stop=True, skip_group_check=True)
                nc.tensor.matmul(oo, Cs(cb_col), colsumO[:, :],
                                 start=False, stop=True, skip_group_check=True)

            def unperm(pb):
                # parity-blocked cols (par*64+k) -> natural (2k+par), as a read view
                return pb.rearrange("p (par k) -> p k par", par=2)

            # small psums share the pt pool (outT only starts much later)
            pc = pt.tile([M, 512], F32, tag="pt")
            fold_mms(pc[0:S, 0:1], 13, 14, bias_sb)
            fold_mms(pc[0:S, 1:2], 5, 8, dbias_sb)
            nc.vector.tensor_copy(cQ[:, :], pc[0:S, 0:1])
            nc.vector.tensor_copy(cD[:, :], pc[0:S, 1:2])
            pq = pt.tile([M, 512], F32, tag="pt")
            field_mms(pq[0:S, 0:M], 3, 9, 10)
            nc.vector.tensor_tensor(QT[:, :], unperm(pq[0:S, 0:M]),
                                    _ap(cQ[:, 0:], [[0, M]]), op=ALU.add)
            pd = pt.tile([M, 512], F32, tag="pt")
            field_mms(pd[0:S, 0:M], 2, 7, 6)
            nc.vector.tensor_tensor(DT[:, :], unperm(pd[0:S, 0:M]),
                                    _ap(cD[:, 0:], [[0, M]]), op=ALU.add)
            pp = pt.tile([M, 512], F32, tag="pt")
            field_mms(pp[0:S, 0:M], 4, 11, 12)
            nc.vector.tensor_copy(PT_sb[:, :], unperm(pp[0:S, 0:M]))
            ppt = pt.tile([M, 512], F32, tag="pt")
            nc.tensor.transpose(ppt[:, 0:S], PT_sb[:, :], ident[0:S, 0:S])
            nc.vector.tensor_copy(P_is.bitcast(F32R), ppt[:, 0:S])
            # contiguous f32r-rounded C0/C1 for the big matmuls
            nc.vector.tensor_copy(c0r_t.bitcast(F32R), Cs(0))
            nc.vector.tensor_copy(c1r_t.bitcast(F32R), Cs(1))

            if until == "small":
                nc.sync.dma_start(out=out_d[0:S, 0:M], in_=QT[:, :])
                nc.sync.dma_start(out=out_d[0:S, M:2 * M], in_=DT[:, :])
                nc.sync.dma_start(out=out_d[0:M, 2 * M:2 * M + S], in_=P_is[:, :])
                return nc

            # ---------------- big matmuls ----------------
            # chunk (c, jp): j in {8c+jp, 8c+2+jp, 8c+4+jp, 8c+6+jp}; psum Z [64, 512]
            # Z free layout: (ip, k, m) -> ip*256 + k*64 + m ; i = 2m+ip
            xe = XT2e.bitcast(F32R)
            xo = XT2o.bitcast(F32R)
            # mm1 rhs stream (ip, pr, m): value X[i=2m+ip, j-of-block, d]
            xe_ipm = xe.rearrange("p (pr m ip) -> p ip pr m", m=S, ip=2)   # [64,2,64,64]
            xo_ipm = xo.rearrange("p (pr m ip) -> p ip pr m", m=S, ip=2)
            # mm2 rhs stream (j2, m): value X[j=8c+2*j2+jp, i-of-block, d]
            xe_mj = xe.rearrange("p (m j2 jp) -> p jp j2 m", j2=S, jp=2)   # [64,2,64,64]
            xo_mj = xo.rearrange("p (m j2 jp) -> p jp j2 m", j2=S, jp=2)
            p_isr = P_is.bitcast(F32R)
            # mmP rhs: indicator delta_{t, i=2m+ip} streamed in (ip, k, m) order
            irep = (identr.rearrange("p (m ip) -> p ip m", ip=2)
                    .unsqueeze(2).broadcast_to([M, 2, 4, S]))
            c0 = c0r_t.bitcast(F32R)
            c1 = c1r_t.bitcast(F32R)

            for cc in range(32):
                c, jp = cc // 2, cc % 2
                xjp = xe_ipm if jp == 0 else xo_ipm
                Z = pz.tile([S, 512], F32, tag="pz")
                # W0: full 512 stream (ip, pr, m)
                nc.tensor.matmul(Z[:, :], c0[:, :], xjp[:, :, 4 * c:4 * c + 4, :],
                                 start=True, stop=False, skip_group_check=True)
                # W1: one mm per i-parity (i-parity selects the XT2 tensor)
                for ip, xmj in ((0, xe_mj), (1, xo_mj)):
                    nc.tensor.matmul(Z[:, ip * 256:(ip + 1) * 256], c1[:, :],
                                     xmj[:, jp, 4 * c:4 * c + 4, :],
                                     start=False, stop=False, skip_group_check=True)
                # P term: + P[i, s] everywhere (indicator rhs, K=128)
                nc.tensor.matmul(Z[:, :], p_isr, irep,
                                 start=False, stop=True, skip_group_check=True)
                # psum -> zz with un-permute: zz col = (8c+2k+jp)*128 + 2m + ip
                zzv = bass.AP(
                    tensor=zz.tensor, offset=zz[:, (8 * c + jp) * M:].offset,
                    ap=[list(zz.ap[0]), [1, 2], [2 * M, 4], [2, S]],
                )
                nc.scalar.copy(out=zzv, in_=Z[:, :])

            if until == "big":
                nc.sync.dma_start(out=out_d[0:S, 0:8192], in_=zz[:, 0:8192])
                nc.sync.dma_start(out=out_d[S:2 * S, 0:8192], in_=zz[:, 8192:16384])
                return nc

            # ---------------- epilogue on zz: +Q, +diag, then transpose+leakyrelu ----------------
            use_dve_q = bool(os.environ.get("KQ_DVE"))
            for w in range(NWAVE):
                sl = slice(w * WJ * M, (w + 1) * WJ * M)
                qte = (QT[:, w * WJ:(w + 1) * WJ]
                       .unsqueeze(2).broadcast_to([S, WJ, M]))
                qeng = nc.vector if use_dve_q else nc.gpsimd
                qeng.tensor_tensor(zz[:, sl], zz[:, sl], qte, op=ALU.add)
                # diag: col j*128 + j = 129*j
                de_o = _ap(zz[:, w * WJ * 129:], [[129, WJ]])
                dte = DT[:, w * WJ:(w + 1) * WJ]
                nc.vector.tensor_tensor(de_o, de_o, dte, op=ALU.add)
                # output transposes: 32 cols -> 4 psum tiles of 8; leaky-relu on copy out
                for g in range(4):
                    ttile = pt.tile([M, 512], F32, tag="pt")
                    for q in range(8):
                        j = w * WJ + g * 8 + q
                        nc.tensor.transpose(
                            ttile[:, q * S:(q + 1) * S],
                            zz[:, j * M:(j + 1) * M],
                            ident[0:S, 0:S],
                        )
                    base = (w * WJ + g * 8) * S
                    osl = out_sb[:, base:base + 512]
                    nc.vector.tensor_scalar_mul(osl, ttile[:, :], 0.01)
                    nc.vector.tensor_max(osl, osl, ttile[:, :])
                nc.sync.dma_start(
                    out=out_d[:, w * WJ * S:(w + 1) * WJ * S],
                    in_=out_sb[:, w * WJ * S:(w + 1) * WJ * S],
                )

            loop_cm.__exit__(None, None, None)

    return nc


_nc_cache = None


def _get_nc():
    global _nc_cache
    if _nc_cache is None:
        _nc_cache = build_nc()
        _nc_cache.compile()
    return _nc_cache


def run_on_cores(inputs, coefs, bias, diag_bias, **spmd_kwargs):
    """Run the SPMD kernel on the 8 cores; returns (out [8,M,M,S], BassKernelResults)."""
    from concourse.bass_utils import run_bass_kernel_spmd

    inputs = np.ascontiguousarray(np.asarray(inputs, dtype=np.float32))
    coefs = np.asarray(coefs, dtype=np.float32)

    # coefs [D, S, B] -> [D, S*B] row-major (free = s*15+b matches kernel views)
    coefs_flat = np.ascontiguousarray(coefs.reshape(D, S * NB))
    bias_col = np.ascontiguousarray(np.asarray(bias, dtype=np.float32).reshape(S, 1))
    dbias_col = np.ascontiguousarray(np.asarray(diag_bias, dtype=np.float32).reshape(S, 1))

    in_maps = []
    for n in range(NCORES):
        in_maps.append({
            "x": np.ascontiguousarray(inputs[n].reshape(M, M * D)),
            "coefs": coefs_flat,
            "biasv": bias_col,
            "dbiasv": dbias_col,
        })

    nc = _get_nc()
    res = run_bass_kernel_spmd(nc, in_maps, list(range(NCORES)), **spmd_kwargs)
    outs = [np.asarray(res.results[n]["out"]).reshape(M, M, S) for n in range(NCORES)]
    return np.stack(outs, axis=0).astype(np.float32), res


def kernel(inputs, nobj, mask, coefs, bias, diag_bias):
    mask = np.asarray(mask, dtype=np.float32)
    out, _ = run_on_cores(inputs, coefs, bias, diag_bias)
    if not np.all(mask == 1.0):
        out = out * mask.reshape(out.shape[0], M, M, 1)
    return out


if __name__ == "__main__":
    nc = build_nc()
    print("built ok")
